# revision 46
# baseline (speedup 1.0000x reference)
"""DeepSwarmLDA Trainium2 kernel.

Math: reference computes
    Xg        = X[:, gene_idx]                        [B, L, G]
    ldas_out  = einsum('blg,lcg->bcl', Xg, lda_W) + lda_b.T
    h         = gelu(ldas_out @ W0.T + b0)            [B, C, 100]
    h         = gelu(h @ W1.T + b1)                   [B, C, 10]
    out       = h @ W2.T + b2                         [B, C, 1]

Everything up to the first gelu is linear in X, so the gather, the per-LDA
classifiers and W0 fold (on host, in float64) into one dense matrix:
    Mfold[n, (c,j)] = sum_{l,g} [gene_idx[l,g]==n] * lda_W[l,c,g] * W0[j,l]
    bias0[(c,j)]    = sum_l lda_b[l,c] * W0[j,l] + b0[j]
giving  h0 = gelu(X @ Mfold + bias0).  The remaining layers act per-c and
fold into block-diagonal matrices W1blk [(c,j),(c,k)] and W2blk [(c,k), c].

Device computes everything transposed (batch on the matmul free axis) so the
contraction dim always sits on SBUF partitions and no transposes are needed:
    h0T[(c,j), b] = gelu(Mfold_tile.T @ XT + bias0)
    h1T[(c,k), b] = gelu(W1blk.T @ h0T + b1)
    outT[c, b]    = W2blk.T @ h1T + b2

Sharding over 8 cores: batch split 4 ways (256 rows each) x C split 2 ways
(c 0-4 / c 5-9, so the aggregation MLP stays core-local). Matmuls run as
float32r (full-rate fp32 PE mode, N=256 free dim).
"""

import numpy as np

import concourse.bass as bass
import concourse.mybir as mybir
from concourse.tile import TileContext
from concourse.tile_rust import add_dep_helper
from concourse.bass_utils import run_bass_kernel_spmd

# Problem shape (hardcoded per contract; kernel.py must be self-contained).
B, NG, L, G, C = 1024, 2000, 1000, 50, 10
J0, J1 = 100, 10

N_CORES = 8
PB, QC = 4, 2              # batch split x c split
BS = B // PB               # 256 batch rows per core
CS = C // QC               # 5 classes per core
KT = 15                    # full 128-row k-tiles (genes 0..1920)
KTAIL = NG - KT * 128      # 80 leftover genes in the tail k-tile
M0 = CS * J0               # 500 h0 channels per core (no padding: m-tile
MW = [128, 128, 128, 116]  # widths; the DMA skips the 12 pad columns)
MOFF = [0, 128, 256, 384]
MT = 4
MC = M0 + BS               # 756 columns per k-tile: [M0 cols | X cols]
M1 = CS * J1               # 50 h1 channels per core
M1P = 64                   # padded
F32 = mybir.dt.float32
F32R = mybir.dt.float32r

# DMA chunk sizes (k-tiles per DMA) for the interleaved M+X operand.
# Concurrent in-flight transfers share SDMA bandwidth round-robin, so
# chunk size does not change aggregate throughput -- but completion sems
# are per-chunk, so SMALL chunks make k-tiles usable earlier (measured:
# [4,4,4,3,1] lost 2.9 us vs [2]*8 purely to chunk-completion granularity).
# All input DMAs ride the two HWDGE rings (lane reuse only costs each
# input chunk its single legal ordering wait); the out-DMA goes via SWDGE
# so it lands on a FRESH lane -- its one wait slot is needed for the
# copy->DMA data dependency.
CHUNKS = [2, 2, 2, 2, 2, 2, 2, 1]
# PE warm-up: dummy matmuls on a zeroed SBUF tile during the DMA lead-in.
# HAM un-throttles after ~3.4us of sustained PE activity (cold MMs run at
# 1.2 GHz = 213 ns for N=256), then re-throttles after >3.4us idle, so the
# warm-up must run right up to the first chunk landing. Over-provisioning
# is nearly free (the stream is DMA-paced, so a warm PE absorbs a small
# backlog), while under-provisioning loses HAM warmth to an idle window.
N_WARM = 26
CW16 = MT * M1P + CS       # f16 consts width: W1blk (256) + W2blk (5)
CW32 = 1                   # f32 consts width: bias1 only


MM_DTYPES = {
    "f32": mybir.dt.float32,
    "f32r": mybir.dt.float32r,
    "bf16": mybir.dt.bfloat16,
    "f16": mybir.dt.float16,
}


def _build_program(act=None, mm="f16"):
    act = act if act is not None else mybir.ActivationFunctionType.Gelu
    mmdt = MM_DTYPES[mm]
    nc = bass.Bass()
    # DRAM layouts are pre-swizzled on host so every DMA is contiguous.
    # The M-operand and X-operand are interleaved per k-tile in ONE tensor so
    # each chunk arrives via a single DMA (the self-loading Matmult encoding
    # only supports one semaphore wait, so every matmul may depend on at
    # most one in-flight transfer):
    #   mx  [128, KT, MC]  mx[p, k, :M0] = Mfold[k*128+p, colshard]
    #                      mx[p, k, M0:] = X[bshard, k*128+p]
    #   mxt [KTAIL+1, MC]  the 80-gene tail k-tile, plus a bias row at
    #                      partition KTAIL: M-part = bias0, X-part = 1.0,
    #                      so the contraction itself adds bias0 and the h0
    #                      activations need no bias operand.
    # Small constants ride in two packed tensors so the h1/out matmul
    # weights are f16 (full-rate PE + FWL) while act biases stay f32:
    #   c16 [128, CW16]: cols 0:256 W1blk swizzled (col m*64+o =
    #                    W1blk[MOFF[m]+p, o]); cols 256:261 W2blk (rows
    #                    0:64, with b2 in row 63 -- matched by a ones row
    #                    memset into h1_sb row 63)
    #   c32 [128, CW32]: col 0 bias1 (rows 0:64); the rest unused
    mx = nc.declare_dram_parameter("mx", [128, KT, MC], mmdt, isOutput=False)
    mxt = nc.declare_dram_parameter("mxt", [KTAIL + 1, MC], mmdt, isOutput=False)
    c16d = nc.declare_dram_parameter("c16", [128, CW16], mmdt, isOutput=False)
    c32d = nc.declare_dram_parameter("c32", [128, CW32], F32, isOutput=False)
    out = nc.declare_dram_parameter("out", [CS, BS], F32, isOutput=True)

    with TileContext(nc) as tc:
        with (
            tc.tile_pool(name="big", bufs=1) as big,
            tc.tile_pool(name="small", bufs=1) as small,
            tc.tile_pool(name="psum", bufs=1, space="PSUM") as psum,
        ):
            sync_targets = []     # observed by SP nops BEFORE the out-DMA
            late_targets = []     # observed after

            # Consts transfer FIRST at the head of the two HWDGE rings:
            # they are tiny (67 KB + 3 KB, ~0.25 us of stream time) and
            # land early enough for the warm-up touch matmuls to observe
            # both lanes without stalling the in-order PE queue. (Putting
            # them on the SWDGE ring instead delayed that ring's mx chunks
            # by ~1.3 us each, measured.)
            c16 = small.tile([128, CW16], mmdt, name="c16")
            sync_targets.append(nc.sync.dma_start(out=c16[:], in_=c16d[:]))
            c32 = small.tile([128, CW32], F32, name="c32")
            sync_targets.append(nc.scalar.dma_start(out=c32[:], in_=c32d[:]))
            # Tail k-tile (80 genes + the bias row): lands EARLY (it is
            # tiny) and its matmuls run early in the accumulation, so the
            # epilogue depends only on the last big chunk.
            tail = small.tile([KTAIL + 1, MC], mmdt, name="mx_tail")
            sync_targets.append(nc.scalar.dma_start(out=tail[:], in_=mxt[:]))

            # Big operand chunks (one tile per chunk so the PE starts as
            # soon as a chunk lands). Triggers cycle over the SP and ACT
            # HWDGE rings plus the SWDGE ring to spread trigger cost.
            mx_ch = []   # per k-tile: (chunk tile, index within chunk)
            c0 = 0
            for ci, csz in enumerate(CHUNKS):
                t = big.tile([128, csz, MC], mmdt, name=f"mx_ch{ci}")
                eng = (nc.sync, nc.scalar)[ci % 2]
                sync_targets.append(
                    eng.dma_start(out=t[:], in_=mx[:, c0 : c0 + csz])
                )
                for j in range(csz):
                    mx_ch.append((t, j))
                c0 += csz
            assert c0 == KT

            # The b2 ones-row lives at partition 64 of h1_sb -- outside the
            # h1 activation's write range 0:64 (no WAR/WAW, and 64 is a
            # legal 32-aligned DVE partition base) -- so the out matmul
            # contracts over 65 rows and its W2blk row 64 (= b2) adds the
            # bias. Emitted BEFORE the w_sb memset so the first warm-up
            # matmul's DVE-lane wait covers both memsets and the out
            # matmul needs no extra wait.
            h1_sb = small.tile([M1P + 1, BS], mmdt, name="h1_sb")
            nc.vector.memset(h1_sb[M1P : M1P + 1, :], 1.0)

            # PE warm-up: zero a scratch tile (DVE, during the preamble),
            # then issue N_WARM dummy matmuls so HAM un-throttles before
            # the first real chunk lands. The last one reads c16 instead:
            # it observes the f16-consts DMA lane mid-warm-up so the h1/out
            # matmuls later carry at most one new wait each.
            w_sb = small.tile([128, BS], mmdt, name="w_sb")
            nc.vector.memset(w_sb[:], 0.0)
            w_ps = psum.tile([128, BS], F32, name="w_ps")
            for _ in range(N_WARM - 1):
                nc.tensor.matmul(
                    w_ps[:], lhsT=w_sb[:, 0:128], rhs=w_sb[:],
                    start=True, stop=True,
                )
            t_ps = psum.tile([1, 1], F32, name="t_ps")
            nc.tensor.matmul(
                t_ps[:], lhsT=c16[:, 0:1], rhs=c16[:, 0:1],
                start=True, stop=True,
            )

            # ACT touch: observe the f32-consts DMA + prewarm the gelu
            # ACT table while the stream runs (a lazy first-use table load
            # costs ~1.3us on the critical tail).
            t_sb = small.tile([128, 1], F32, name="t_sb")
            nc.scalar.activation(
                t_sb[:], c32[:, 0:1], mybir.ActivationFunctionType.Identity
            )
            nc.scalar.activation(t_sb[:], c32[:, 0:1], act)

            # h0T = Mfold.T @ XT accumulated over 15 full k-tiles plus the
            # 81-row tail (80 genes + bias0 row). One PSUM bank per m-tile:
            # interleaved accumulation groups must NOT share a bank
            # (start=True clears the whole bank, wiping the sibling
            # group's first k-tile -- measured as a 25% error).
            #
            # m3's lhsT reads a full 128 columns, spilling 12 columns into
            # the X region: PSUM rows 116:128 get defined garbage that
            # nothing downstream reads.
            h0_ps = [
                psum.tile([128, BS], F32, name=f"h0_ps{m}") for m in range(MT)
            ]
            korder = [0, 1, "tail"] + list(range(2, KT))
            for ki, k in enumerate(korder):
                if k == "tail":
                    ch = tail[:]
                else:
                    t, j = mx_ch[k]
                    ch = t[:, j]
                for m in range(MT):
                    nc.tensor.matmul(
                        h0_ps[m][:],
                        lhsT=ch[:, MOFF[m] : MOFF[m] + 128],
                        rhs=ch[:, M0:],
                        start=(ki == 0),
                        stop=(ki == KT),
                    )

            # gelu(h0), PSUM -> SBUF (f16) on the scalar engine (no bias:
            # folded into the matmul via the bias0 row).
            h0_sb = [
                small.tile([128, BS], mmdt, name=f"h0_sb{m}") for m in range(MT)
            ]
            for m in range(MT):
                nc.scalar.activation(h0_sb[m][:], h0_ps[m][:], act)

            # h1T = W1blk.T @ h0T (contraction = 4 m-tiles).
            h1_ps = psum.tile([M1P, BS], F32, name="h1_ps")
            for m in range(MT):
                nc.tensor.matmul(
                    h1_ps[:],
                    lhsT=c16[0 : MW[m], m * M1P : (m + 1) * M1P],
                    rhs=h0_sb[m][0 : MW[m], :],
                    start=(m == 0),
                    stop=(m == MT - 1),
                )
            nc.scalar.activation(
                h1_sb[0:M1P, :],
                h1_ps[:],
                act,
                bias=c32[0:M1P, 0:1],
            )

            # outT = W2blk.T @ h1T (+ b2 via the ones-row at partition 64).
            o_ps = psum.tile([CS, BS], F32, name="o_ps")
            sync_targets.append(nc.tensor.matmul(
                o_ps[:],
                lhsT=c16[0 : M1P + 1, MT * M1P : MT * M1P + CS],
                rhs=h1_sb[:],
                start=True,
                stop=True,
            ))
            # PSUM -> SBUF copy on the ACT engine (free right after the h1
            # gelu); the out-DMA then carries exactly one wait (this copy)
            # on its fresh SWDGE lane.
            o_sb = small.tile([CS, BS], F32, name="o_sb")
            late_targets.append(nc.scalar.activation(
                o_sb[:], o_ps[:], mybir.ActivationFunctionType.Copy
            ))

            # The kernel-tail drain puts a wait on every proc SP has not
            # observed, and its encoding holds only a few waits. Chain SP
            # NOPs, one sync dep each, so SP observes every DMA lane and
            # engine tick incrementally and the drain has nothing left.
            # Observing all input-DMA lanes BEFORE issuing the output DMA
            # also elides the out-DMA's same-lane ordering wait (HWDGE
            # waits execute on the issuing sequencer), keeping it at the
            # one-wait encoding limit regardless of input DMA count.
            for t in sync_targets:
                nop = nc.sync.nop()
                add_dep_helper(
                    nop.ins, t.ins, sync=True, reason="spread drain waits"
                )

            late_targets.append(nc.gpsimd.dma_start(out=out[:], in_=o_sb[:]))
            for t in late_targets:
                nop = nc.sync.nop()
                add_dep_helper(
                    nop.ins, t.ins, sync=True, reason="spread drain waits"
                )

    return nc


def _fold_weights(gene_idx, lda_W, lda_b, W0, b0):
    """Fold gather + per-LDA linear + W0 into Mfold [NG, C, J0] and
    bias0 [C, J0], computed in float64."""
    lda_W64 = lda_W.astype(np.float64)
    W064 = W0.astype(np.float64)
    # A[l, n, c] = sum_g [gene_idx[l,g]==n] * lda_W[l,c,g]
    A = np.zeros((L, NG, C), dtype=np.float64)
    l_rep = np.repeat(np.arange(L), G)
    np.add.at(A, (l_rep, gene_idx.ravel()), lda_W64.transpose(0, 2, 1).reshape(L * G, C))
    # Mfold[n, c, j] = sum_l A[l, n, c] * W0[j, l]
    Mfold = (W064 @ A.reshape(L, NG * C)).reshape(J0, NG, C).transpose(1, 2, 0)
    bias0 = np.einsum("lc,jl->cj", lda_b.astype(np.float64), W064) + b0.astype(
        np.float64
    )
    return Mfold, bias0


_prog_cache = {}


def _get_program(act=None, mm="f32r"):
    key = ("nc", act, mm)
    if key not in _prog_cache:
        _prog_cache[key] = _build_program(act, mm)
    return _prog_cache[key]


def _round_tf32(a):
    """Round fp32 array to the TF32 grid (10-bit mantissa, RNE)."""
    u = np.ascontiguousarray(a, dtype=np.float32).view(np.uint32)
    lsb = (u >> 13) & np.uint32(1)
    u2 = (u + np.uint32(0x0FFF) + lsb) & np.uint32(0xFFFFE000)
    return u2.view(np.float32)


def _mm_convert(a, mm):
    if mm == "f32":
        return a
    if mm == "f32r":
        return _round_tf32(a)
    if mm == "bf16":
        import ml_dtypes

        return a.astype(ml_dtypes.bfloat16)
    if mm == "f16":
        return a.astype(np.float16)
    raise ValueError(mm)


def _prepare_in_maps(X, gene_idx, lda_W, lda_b, W0, b0, W1, b1, W2, b2, mm="f32r"):
    X = np.asarray(X, dtype=np.float32)
    gene_idx = np.asarray(gene_idx)
    lda_W = np.asarray(lda_W, dtype=np.float32)
    lda_b = np.asarray(lda_b, dtype=np.float32)
    W0 = np.asarray(W0, dtype=np.float32)
    b0 = np.asarray(b0, dtype=np.float32)
    W1 = np.asarray(W1, dtype=np.float32)
    b1 = np.asarray(b1, dtype=np.float32)
    W2 = np.asarray(W2, dtype=np.float32)
    b2 = np.asarray(b2, dtype=np.float32)

    Mfold, bias0 = _fold_weights(gene_idx, lda_W, lda_b, W0, b0)

    # Per-C-half weight shards.
    KFULL = KT * 128  # 1920 genes covered by full k-tiles
    mf_maps, mt_maps, c16_maps, c32_maps = [], [], [], []
    for ch in range(QC):
        cs = slice(ch * CS, (ch + 1) * CS)
        # Mfold columns for this half, c-major flatten [NG, M0]: full
        # k-tiles swizzled to [128, KT, M0], tail rows + bias0 row kept
        # separately.
        mcols = Mfold[:, cs, :].reshape(NG, M0).astype(np.float32)
        mf_maps.append(np.ascontiguousarray(
            mcols[:KFULL].reshape(KT, 128, M0).transpose(1, 0, 2)))
        mtail = np.empty((KTAIL + 1, M0), dtype=np.float32)
        mtail[:KTAIL] = mcols[KFULL:]
        mtail[KTAIL] = bias0[cs, :].reshape(M0)
        mt_maps.append(mtail)

        # Packed f16 matmul weights [128, CW16]: W1blk swizzle + W2blk
        # (with b2 in row 63 to pair with the h1_sb ones-row).
        c16_arr = np.zeros((128, CW16), dtype=np.float32)
        w1blk = np.zeros((M0, M1P), dtype=np.float32)
        for c in range(CS):
            w1blk[c * J0 : (c + 1) * J0, c * J1 : (c + 1) * J1] = W1.T
        for m in range(MT):
            c16_arr[: MW[m], m * M1P : (m + 1) * M1P] = (
                w1blk[MOFF[m] : MOFF[m] + MW[m], :]
            )
        w2blk = np.zeros((M1P + 1, CS), dtype=np.float32)
        for c in range(CS):
            w2blk[c * J1 : (c + 1) * J1, c] = W2[0]
        w2blk[M1P, :] = b2[0]
        c16_arr[: M1P + 1, MT * M1P : MT * M1P + CS] = w2blk
        c16_maps.append(_mm_convert(c16_arr, mm))

        # Packed f32 act biases [128, CW32]: bias1 only.
        c32_arr = np.zeros((128, CW32), dtype=np.float32)
        c32_arr[:M1, 0] = np.tile(b1, CS)
        c32_maps.append(c32_arr)

    # Batch shards, transposed: full k-tiles swizzled to [128, KT, BS],
    # tail rows + the all-ones bias row kept separately.
    xt_maps, xtail_maps = [], []
    for bq in range(PB):
        xs = X[bq * BS : (bq + 1) * BS, :].T  # [NG, BS]
        xt_maps.append(np.ascontiguousarray(
            xs[:KFULL].reshape(KT, 128, BS).transpose(1, 0, 2)))
        xtail = np.empty((KTAIL + 1, BS), dtype=np.float32)
        xtail[:KTAIL] = xs[KFULL:]
        xtail[KTAIL] = 1.0
        xtail_maps.append(xtail)

    in_maps = []
    for core in range(N_CORES):
        bq, ch = core % PB, core // PB
        mxa = np.concatenate([mf_maps[ch], xt_maps[bq]], axis=2)
        mxt = np.concatenate([mt_maps[ch], xtail_maps[bq]], axis=1)
        in_maps.append({
            "mx": _mm_convert(np.ascontiguousarray(mxa), mm),
            "mxt": _mm_convert(np.ascontiguousarray(mxt), mm),
            "c16": c16_maps[ch],
            "c32": c32_maps[ch],
        })
    return in_maps


def _assemble(core_outs):
    out = np.empty((B, C, 1), dtype=np.float32)
    for core in range(N_CORES):
        bq, ch = core % PB, core // PB
        o = core_outs[core]  # [CS, BS]
        out[bq * BS : (bq + 1) * BS, ch * CS : (ch + 1) * CS, 0] = o.T
    return out


MM_MODE = "f16"


def kernel(X, gene_idx, lda_W, lda_b, W0, b0, W1, b1, W2, b2, _trace=False,
           _mm=None):
    mm = _mm or MM_MODE
    in_maps = _prepare_in_maps(
        X, gene_idx, lda_W, lda_b, W0, b0, W1, b1, W2, b2, mm=mm
    )
    nc = _get_program(mm=mm)
    res = run_bass_kernel_spmd(
        nc, in_maps, core_ids=list(range(N_CORES)), trace=_trace
    )
    out = _assemble([res.results[c]["out"] for c in range(N_CORES)])
    if _trace:
        return out, res
    return out

